# revision 2
# baseline (speedup 1.0000x reference)
"""Trainium2 Bass kernel for the CapibaraByte recurrent-scan problem.

Reference computation (B=128, T=1024, D_IN=256, H=2048):
    conv = einsum('btd,dh->bth', x, W_conv)
    step:  s <- 0.9*s + 0.1*gelu(s @ W_state + conv[:,t] + bias)
    out = (s @ W_state + bias, s)

Strategy (v2): data-parallel over batch across 8 cores (B_local=16/core);
the scan runs fully on-core with zero cross-core traffic.

Per-step GEMM is state-stationary with 4-way PE column tiling (four 16-wide
stationary tiles on disjoint 32-col PE strips, each streaming its own
512-wide slice of W concurrently).  The conv projection x_t @ W_conv and
the bias add are FUSED into the same PSUM accumulation as extra
contraction tiles (k=16,17 from x_t, k=18 a ones-vector against a
bias/128 row), so there is no separate conv phase at all.

The state lives in a permuted [h, b] layout (col = 64*(tau%4) +
16*(tau//4) + b for h-tile tau) so that the per-step transpose back to
[h, b] needs only four [64,128] PE transposes: the four PSUM column
groups are evicted into vertically-stacked 16-row stripes of one [64,
2048] SBUF tile, and each [64,128] slice of that tile transposes into
one contiguous [128,64] block of the new state.

The tail (gelu -> 0.1*g + 0.9*s -> bf16 cast) is split per transposed
block so the next step's matmul rounds unblock block-by-block; the x/bias
rounds of each step are issued first since they need no state.  All
matmul operands are bf16 (fp32 PSUM accumulate); an fp32 master copy of
the state keeps the blend exact.
"""

import sys

for _p in ("/opt/trn_rl_repo",):
    if _p not in sys.path:
        sys.path.insert(0, _p)

import numpy as np
import ml_dtypes

import concourse.bass as bass
import concourse.tile as tile
from concourse import bacc, mybir
from concourse.bass import ds
from concourse.bass_utils import run_bass_kernel_spmd

AFT = mybir.ActivationFunctionType
ALU = mybir.AluOpType
F32 = mybir.dt.float32
BF16 = mybir.dt.bfloat16

B, T_FULL, D_IN, H = 128, 1024, 256, 2048
NCORES = 8
BL = B // NCORES            # 16 batch rows per core
KT = H // 128               # 16 state contraction tiles
MT = H // 128               # 16 output h-tiles
UPDATE = 0.1

# permuted state column layout: col(tau, b) = 64*(tau%4) + 16*(tau//4) + b
def _tau_col(tau):
    return 64 * (tau % 4) + 16 * (tau // 4)


def build(T_steps=T_FULL, U=8, with_bias=False, act=AFT.Gelu_apprx_tanh):
    """Build the Bacc graph. Body handles 2*U steps; T_steps % (2U) == 0."""
    assert T_steps % (2 * U) == 0
    nc = bacc.Bacc("TRN2", target_bir_lowering=False, debug=False,
                   num_devices=NCORES)

    NK = 19 if with_bias else 18        # contraction tiles incl. x (+bias)
    UB = U * BL                         # x cols per k-chunk per U-block

    # x transposed, padded by one U-block: xT[kc, p, t*BL+b] = x[b,t,128kc+p]
    xT_d = nc.dram_tensor("xT", [2, 128, (T_steps + U) * BL], BF16,
                          kind="ExternalInput").ap()
    # W rows: k<16 state, k=16,17 conv, k=18 bias/128 (if with_bias)
    w_d = nc.dram_tensor("w_arr", [128, NK * H], BF16,
                         kind="ExternalInput").ap()
    biasT_d = nc.dram_tensor("bias_bcT", [128, MT * BL], F32,
                             kind="ExternalInput").ap()
    ident_d = nc.dram_tensor("ident", [128, 128], F32,
                             kind="ExternalInput").ap()
    outT_d = nc.dram_tensor("outT", [128, MT * BL], F32,
                            kind="ExternalOutput").ap()
    stT_d = nc.dram_tensor("stT", [128, MT * BL], F32,
                           kind="ExternalOutput").ap()

    with tile.TileContext(nc) as tc:
        with (
            tc.tile_pool(name="persist", bufs=1) as persist,
            tc.tile_pool(name="g64p", bufs=2) as g64p,
            tc.tile_pool(name="work", bufs=2) as work,
            tc.tile_pool(name="ps", bufs=8, space="PSUM") as psp,
        ):
            # ---- resident tensors ----
            w_sb = persist.tile([128, NK * H], BF16, tag="w_sb")
            nc.sync.dma_start(w_sb[:], w_d[:])
            biasT_sb = persist.tile([128, MT * BL], F32, tag="biasT_sb")
            nc.sync.dma_start(biasT_sb[:], biasT_d[:])
            ident_sb = persist.tile([128, 128], F32, tag="ident_sb")
            nc.sync.dma_start(ident_sb[:], ident_d[:])
            ones_sb = persist.tile([128, BL], BF16, tag="ones_sb")
            nc.vector.memset(ones_sb[:], 1.0)

            # state ping-pong (permuted layout, see _tau_col)
            stT_bf = [persist.tile([128, MT * BL], BF16, tag=f"stbf{j}")
                      for j in range(2)]
            stT_f32 = [persist.tile([128, MT * BL], F32, tag=f"stf{j}")
                       for j in range(2)]
            for j in range(2):
                nc.vector.memset(stT_bf[j][:], 0.0)
                nc.vector.memset(stT_f32[j][:], 0.0)

            # x staging ping-pong (persistent; preloaded before the loop)
            xA = persist.tile([128, 2 * UB], BF16, tag="xA")
            xB = persist.tile([128, 2 * UB], BF16, tag="xB")

            def dma_x(dst, blk_i):
                """Load U-block blk_i (runtime index ok) into dst."""
                for kc in range(2):
                    nc.sync.dma_start(
                        dst[:, kc * UB:(kc + 1) * UB],
                        xT_d[kc, :, ds(blk_i * UB, UB)])

            ROW_ORDER = [k for j in range(4) for k in range(j, KT, 4)]

            def mm_rounds(sus, dst_bf, xsb, u, rounds):
                """Issue MM rounds (4 col-tiled matmuls each) for one step.

                sus: list of 4 psum tiles; dst_bf: state bf16 tile;
                xsb: x staging tile; u: step index within the U-block;
                rounds: subset of contraction-tile indices to issue.
                """
                for k in rounds:
                    if k < KT:
                        lhs = dst_bf[:, _tau_col(k):_tau_col(k) + BL]
                    elif k < KT + 2:
                        kc = k - KT
                        lhs = xsb[:, kc * UB + u * BL:kc * UB + (u + 1) * BL]
                    else:
                        lhs = ones_sb[:]
                    for g in range(4):
                        nc.tensor.matmul(
                            sus[g][32 * g:32 * g + BL, :],
                            lhsT=lhs,
                            rhs=w_sb[:, k * H + 512 * g:k * H + 512 * (g + 1)],
                            start=(k == rounds[0]), stop=(k == rounds[-1]),
                            tile_position=(0, 32 * g),
                        )

            def alloc_sus():
                return [psp.tile([128, 512], F32, tag="ps", name=f"su{g}")
                        for g in range(4)]

            def tail(sus, cur, nxt, tmp, final=False):
                """Evict+transpose+gelu+blend: sus -> state[nxt].

                If final: skip gelu/blend; write out = su^T + biasT to work
                tile and return it.
                """
                g64 = g64p.tile([64, H], F32, tag="g64")
                for g in range(4):
                    src = sus[g][32 * g:32 * g + BL, :]
                    dstp = g64[16 * g:16 * g + BL, :]
                    if g % 2 == 0:
                        nc.vector.tensor_copy(dstp, src)
                    else:
                        nc.scalar.copy(dstp, src)
                pT = psp.tile([128, 512], F32, tag="ps", name="pT")
                for tpp in range(4):
                    nc.tensor.matmul(
                        pT[:, 64 * tpp:64 * (tpp + 1)],
                        lhsT=g64[:, 128 * tpp:128 * (tpp + 1)],
                        rhs=ident_sb[0:64, 0:64],
                        is_transpose=True, start=True, stop=True,
                    )
                if final:
                    outf = work.tile([128, MT * BL], F32, tag="outf")
                    nc.vector.tensor_tensor(
                        outf[:], pT[:, 0:MT * BL], biasT_sb[:], ALU.add)
                    return outf
                # per-block: gelu -> 0.1*g + 0.9*s -> bf16, block 0 alone
                # (unblocks next step's first rounds early), rest coarse.
                gsb = work.tile([128, MT * BL], F32, tag="gsb")
                for lo, hi in ((0, 64), (64, 256)):
                    nc.scalar.activation(gsb[:, lo:hi], pT[:, lo:hi], act)
                    nc.vector.scalar_tensor_tensor(
                        stT_f32[nxt][:, lo:hi], gsb[:, lo:hi], UPDATE,
                        tmp[:, lo:hi], ALU.mult, ALU.add)
                    nc.vector.tensor_copy(
                        stT_bf[nxt][:, lo:hi], stT_f32[nxt][:, lo:hi])
                return None

            X_ROUNDS = list(range(KT, NK))

            def half_block(xsb, base_par, carry):
                """U steps using x staging tile xsb. carry = pending tail
                state (sus, cur, nxt, tmp) from the previous step."""
                for u in range(U):
                    cur = (base_par + u) % 2
                    nxt = 1 - cur
                    sus = alloc_sus()
                    # x/bias rounds first: no state dependency
                    mm_rounds(sus, stT_bf[cur], xsb, u, X_ROUNDS)
                    # previous step's tail (unblocks state rounds below)
                    if carry is not None:
                        tail(*carry)
                    # tmp = 0.9 * s  for THIS step's tail (s = state[cur])
                    tmp = work.tile([128, MT * BL], F32, tag="tmp")
                    nc.vector.tensor_scalar_mul(tmp[:], stT_f32[cur][:],
                                                1.0 - UPDATE)
                    mm_rounds(sus, stT_bf[cur], xsb, u, ROW_ORDER)
                    carry = (sus, cur, nxt, tmp)
                return carry

            n_iters = T_steps // (2 * U)
            dma_x(xA, 0)  # preload first half-block

            carry = None
            with tc.For_i(0, n_iters, 1,
                          hint_engines=(mybir.EngineType.PE,)) as i:
                dma_x(xB, 2 * i + 1)
                carry = half_block(xA, 0, carry)
                dma_x(xA, 2 * i + 2)      # next iteration (xT is padded)
                carry = half_block(xB, U, carry)
                tail(*carry)              # can't carry across back-edge
                carry = None

            # ---- final output = state @ W_state + bias ----
            cur = 0                       # 2U steps per body, even
            sus = alloc_sus()
            fin_rounds = ROW_ORDER + ([KT + 2] if with_bias else [])
            mm_rounds(sus, stT_bf[cur], xA, 0, fin_rounds)
            outf = tail(sus, cur, 1, None, final=True)
            nc.sync.dma_start(outT_d[:], outf[:])
            nc.sync.dma_start(stT_d[:], stT_f32[cur][:])

    nc.compile()
    return nc


def host_inputs(x, W_state, W_conv, bias, T_steps=T_FULL, U=8):
    """Per-core input dicts. x: (B, T_steps, D_IN) f32."""
    bf = ml_dtypes.bfloat16
    with_bias = bool(np.any(bias))
    NK = 19 if with_bias else 18
    # W rows: w[p, k*H+n] = W_aug[128k+p, n]
    w_arr = np.empty((128, NK * H), np.float32)
    for k in range(KT):
        w_arr[:, k * H:(k + 1) * H] = W_state[128 * k:128 * (k + 1)]
    for kc in range(2):
        w_arr[:, (KT + kc) * H:(KT + kc + 1) * H] = \
            W_conv[128 * kc:128 * (kc + 1)]
    if with_bias:
        w_arr[:, 18 * H:19 * H] = np.broadcast_to(bias / 128.0, (128, H))
    w_arr = w_arr.astype(bf)
    # biasT in permuted layout: col(tau,b) -> bias[128*tau + p]
    biasT = np.empty((128, MT * BL), np.float32)
    for tau in range(MT):
        c = _tau_col(tau)
        biasT[:, c:c + BL] = bias[128 * tau:128 * (tau + 1), None]
    ident = np.eye(128, dtype=np.float32)

    Tpad = T_steps + U
    in_maps = []
    for c in range(NCORES):
        xs = x[c * BL:(c + 1) * BL]          # [BL, T, D]
        # xT[kc, p, t*BL+b] = xs[b, t, kc*128+p], padded to Tpad
        xT = np.zeros((2, 128, Tpad * BL), bf)
        xT[:, :, :T_steps * BL] = (
            xs.reshape(BL, T_steps, 2, 128).transpose(2, 3, 1, 0)
            .reshape(2, 128, T_steps * BL)).astype(bf)
        in_maps.append({
            "xT": xT, "w_arr": w_arr,
            "bias_bcT": biasT, "ident": ident,
        })
    return in_maps


def _unpermute(arr):
    """[128, MT*BL] permuted -> [BL, H]: col = 64*(tau%4)+16*(tau//4)+b."""
    # arr[p, 64*tpp + 16*s + b] = val[b, 512*s + 128*tpp + p]
    a4 = arr.reshape(128, 4, 4, BL)          # [p, tpp, s, b]
    return np.ascontiguousarray(a4.transpose(3, 2, 1, 0)).reshape(BL, H)


def gather_outputs(results):
    out = np.empty((B, H), np.float32)
    st = np.empty((B, H), np.float32)
    for c, r in enumerate(results):
        out[c * BL:(c + 1) * BL] = _unpermute(r["outT"])
        st[c * BL:(c + 1) * BL] = _unpermute(r["stT"])
    return out, st


_NC_CACHE = {}


def _get_nc(T_steps=T_FULL, U=8, with_bias=False):
    key = (T_steps, U, with_bias)
    if key not in _NC_CACHE:
        _NC_CACHE[key] = build(T_steps, U, with_bias)
    return _NC_CACHE[key]


def kernel(x, W_state, W_conv, bias):
    x = np.asarray(x, np.float32)
    W_state = np.asarray(W_state, np.float32)
    W_conv = np.asarray(W_conv, np.float32)
    bias = np.asarray(bias, np.float32)
    nc = _get_nc(T_FULL, 8, bool(np.any(bias)))
    in_maps = host_inputs(x, W_state, W_conv, bias, T_FULL, 8)
    res = run_bass_kernel_spmd(nc, in_maps, list(range(NCORES)))
    return gather_outputs(res.results)
